# revision 18
# baseline (speedup 1.0000x reference)
"""Trainium2 Bass kernel v2 for the 4-layer GCN + mesh-unpool network,
8 NeuronCores, dst-sharded graph parallelism.

vs v1 (baseline): per-column indirect DMA gathers (994ns Q7 emission per
128 rows -> ~14ms) are replaced by bulk InstDMAGatherAnt gathers
(~8k rows per instruction, round-robin over 4 SWDGE queues), with bf16
tables packed K nodes per 256B+ row (pure reinterpretation of the
row-major [V, F] table).  W3/W4 are pushed through the aggregation
(linearity), eliminating both AllToAll exchanges and the fetch/build
phases; unpools become host-side index composition.

Per layer l (dst shard on each core): padded-CSR slot grids over
in-degree-sorted nodes; slots gathered from the bf16 table (row idx//K,
sub-node idx%K selected by a DVE mask), weighted by dis_l[src] (and
killed for pad slots), segment-reduced, scaled by dis_l[dst], matmul'd
(W1/W2; identity collapse for L3/L4 which gather pre-multiplied
tables), relu'd, stored bf16, AllGathered into the next table.
"""
import sys
sys.path.insert(0, "/opt/trn_rl_repo")

import numpy as np
import ml_dtypes

NC = 8
P = 128
NQ = 4            # SWDGE queues
MAXIDX = 8192     # idx per dma_gather instruction (HW-validated)


def pad_to(x, m):
    return (x + m - 1) // m * m


# ----------------------------------------------------------------------------
# host-side planning
# ----------------------------------------------------------------------------

def make_dis(edge_index, n):
    deg = np.bincount(edge_index[1], minlength=n).astype(np.float64) + 1.0
    return (1.0 / np.sqrt(deg)).astype(np.float32)


def plan_agg(dst_old, slot_row, n, self_row, slot_weight, self_weight, G,
             col_budget=64):
    """Padded-CSR plan (degree-sorted, program-unified across cores).

    Returns per-core idx [P, Ctot] int32 (table row ids), w [P, Ctot]
    fp32, groups [(Gg, kt)], sigma (old id -> global new row), dis-dst
    columns come separately.
    """
    shard = n // NC
    shardP = pad_to(shard, P)
    ntiles = shardP // P
    c_of = dst_old // shard

    percore = []
    for c in range(NC):
        m = c_of == c
        dl = dst_old[m] - c * shard
        deg = np.bincount(dl, minlength=shard)
        perm = np.argsort(-deg, kind="stable")
        inv = np.empty_like(perm); inv[perm] = np.arange(shard)
        percore.append((m, dl, deg, perm, inv))

    def span_kt(lo_t, Gg):
        lo, hi = lo_t * P, min((lo_t + Gg) * P, shard)
        kt = 0
        for c in range(NC):
            deg_new = percore[c][2][percore[c][3]]
            if hi > lo:
                kt = max(kt, int(deg_new[lo:hi].max()))
        return kt + 1

    groups = []
    t = 0
    while t < ntiles:
        Gg = min(G, ntiles - t)
        kt = span_kt(t, Gg)
        while Gg > 1 and Gg * kt > col_budget:
            Gg = max(1, min(Gg - 1, col_budget // kt))
            kt = span_kt(t, Gg)
        groups.append((Gg, kt))
        t += Gg

    colbase = np.zeros(ntiles + 1, np.int64)
    kts_tile = []
    for (Gg, kt) in groups:
        kts_tile += [kt] * Gg
    for t in range(ntiles):
        colbase[t + 1] = colbase[t] + kts_tile[t]
    Ctot = int(colbase[-1])

    out = {"groups": groups, "idx": [], "w": [], "perm": [],
           "shardP": shardP, "Ctot": Ctot}
    sigma = np.empty(n, np.int64)
    for c in range(NC):
        m, dl, deg, perm, inv = percore[c]
        sigma[c * shard:(c + 1) * shard] = c * shardP + inv
        sr = slot_row[m]
        sw = slot_weight[m]
        nd = inv[dl]
        order = np.argsort(nd, kind="stable")
        nd_s = nd[order]; sr_s = sr[order]
        deg_new = deg[perm]
        starts = np.zeros(shard + 1, np.int64)
        np.cumsum(deg_new, out=starts[1:])
        idx = np.zeros((P, Ctot), np.int64)
        w = np.zeros((P, Ctot), np.float32)
        r = np.arange(len(nd_s)) - starts[nd_s]
        pp_ = nd_s % P
        tt_ = nd_s // P
        cols = colbase[tt_] + r
        idx[pp_, cols] = sr_s
        w[pp_, cols] = sw[order]
        v = np.arange(shard)
        scols = colbase[v // P] + deg_new[v]
        idx[v % P, scols] = self_row[perm + c * shard]
        w[v % P, scols] = self_weight[perm + c * shard]
        out["idx"].append(idx)
        out["w"].append(w)
        out["perm"].append(perm)
    out["sigma"] = sigma
    return out


def tile_cols(vec_percore, shardP):
    outs = []
    for v in vec_percore:
        a = np.zeros(shardP, np.float32)
        a[:len(v)] = v
        outs.append(a.reshape(shardP // P, P).T.copy())
    return outs


def wrap16(idx_cols):
    """[P, C] column-major slot grid -> dma_gather idx layout.

    Slot i (= c*128 + p) must sit at (partition i%16, col i//16),
    replicated across the 8 16-partition groups. Returns [P, C*8] int16.
    """
    Pp, C = idx_cols.shape
    flat = idx_cols.T.reshape(-1)              # slot i order
    n = flat.shape[0]
    w = flat.reshape(n // 16, 16).T            # [16, n/16]
    return np.tile(w, (8, 1)).astype(np.int16) # [128, n/16]


def split_kq(idx, K):
    return (idx // K).astype(np.int64), (idx % K).astype(np.float32)


def prep(inputs, G1=16, G2=2, G3=4, G4=16, GF=8):
    x = np.asarray(inputs["x"], np.float32)
    W = [np.asarray(inputs[f"W{i}"], np.float32) for i in (1, 2, 3, 4)]
    ei = [np.asarray(inputs[f"edge_index{i}"]).astype(np.int64) for i in range(4)]
    u = [np.asarray(inputs[f"unpool{i}"]).astype(np.int64) for i in (1, 2, 3, 4)]
    n = [x.shape[0], len(u[0]), len(u[1]), len(u[2])]
    nout = len(u[3])
    dis = [make_dis(ei[l], n[l]) for l in range(4)]

    # L1: table rows = original x ids (dis1 folded into table)
    L1 = plan_agg(ei[0][1], ei[0][0], n[0], self_row=np.arange(n[0]), G=G1,
                  slot_weight=np.ones(ei[0].shape[1], np.float32),
                  self_weight=np.ones(n[0], np.float32), col_budget=128)
    s1 = L1["sigma"]
    # L2: rows in T2 (=A1, global new order), composed via u1
    L2 = plan_agg(ei[1][1], s1[u[0][ei[1][0]]], n[1], self_row=s1[u[0]], G=G2,
                  slot_weight=dis[1][ei[1][0]], self_weight=dis[1])
    s2 = L2["sigma"]
    # L3: rows in T3 (=B2 = A2@W3), composed via u2
    L3 = plan_agg(ei[2][1], s2[u[1][ei[2][0]]], n[2], self_row=s2[u[1]], G=G3,
                  slot_weight=dis[2][ei[2][0]], self_weight=dis[2])
    s3 = L3["sigma"]
    # L4: rows in T4 (=B3 = A3@W4), composed via u3
    L4 = plan_agg(ei[3][1], s3[u[2][ei[3][0]]], n[3], self_row=s3[u[2]], G=G4,
                  slot_weight=dis[3][ei[3][0]], self_weight=dis[3],
                  col_budget=128)
    s4 = L4["sigma"]

    # dis-dst columns (new-local order per core)
    dis_dst = []
    for l, L in enumerate((L1, L2, L3, L4)):
        sh = n[l] // NC
        dis_dst.append(tile_cols(
            [dis[l][c * sh + L["perm"][c]] for c in range(NC)], L["shardP"]))

    # per-layer packing K (nodes per 256B+ table row, bf16)
    Ks = {"1": 16, "2": 4, "3": 8, "4": 16, "F": 32}
    Fs = {"1": 4, "2": 32, "3": 32, "4": 4, "F": 4}

    # final unpool is applied on the host during unshard/reassembly
    shf = nout // NC
    shfP = pad_to(shf, P)
    CF = shfP // P
    frows = s4[u[3]]

    meta = dict(
        n=n, nout=nout, shf=shf, shfP=shfP, CF=CF, Ks=Ks, Fs=Fs,
        L1=dict(groups=L1["groups"], shardP=L1["shardP"], C=L1["Ctot"],
                fout=32, wmat="w1", relu=True),
        L2=dict(groups=L2["groups"], shardP=L2["shardP"], C=L2["Ctot"],
                fout=64, wmat="w2", relu=True),
        L3=dict(groups=L3["groups"], shardP=L3["shardP"], C=L3["Ctot"],
                fout=32, wmat=None, relu=True),
        L4=dict(groups=L4["groups"], shardP=L4["shardP"], C=L4["Ctot"],
                fout=4, wmat=None, relu=True),
        T1_rows=pad_to(n[0], Ks["1"]) // Ks["1"],
        T2_rows=NC * L1["shardP"] // Ks["2"],
        T3_rows=NC * L2["shardP"] // Ks["3"],
        T4_rows=NC * L3["shardP"] // Ks["4"],
        T5_rows=NC * L4["shardP"] // Ks["F"],
    )

    # ---- per-core inputs ----
    # T1: x * dis1, padded to 4 feats, bf16, row-major [V,4] (viewed packed)
    V1 = pad_to(n[0], Ks["1"])
    T1 = np.zeros((V1, 4), np.float32)
    T1[:n[0], :3] = x * dis[0][:, None]

    W1p = np.zeros((4, 32), np.float32); W1p[:3] = W[0]
    W4p = np.zeros((32, 4), np.float32); W4p[:, :3] = W[3]

    def blkdiag(Wm, G):
        fi, fo = Wm.shape
        B = np.zeros((G * fi, G * fo), np.float32)
        for g in range(G):
            B[g * fi:(g + 1) * fi, g * fo:(g + 1) * fo] = Wm
        return B

    ident = np.eye(P, dtype=np.float32)
    qio = np.tile(np.arange(32, dtype=np.float32)[None, :], (128, 1)).astype(ml_dtypes.bfloat16)

    in_maps = []
    for c in range(NC):
        i1, q1 = split_kq(L1["idx"][c], Ks["1"])
        i2, q2 = split_kq(L2["idx"][c], Ks["2"])
        i3, q3 = split_kq(L3["idx"][c], Ks["3"])
        i4, q4 = split_kq(L4["idx"][c], Ks["4"])
        m = {
            "T1": T1, "ident": ident, "qio": qio,
            "w1blk": blkdiag(W1p, G1), "w2blk": blkdiag(W[1], G2),
            "w3blk": blkdiag(W[2], G2),
            "idx1": wrap16(i1), "q1": q1.astype(ml_dtypes.bfloat16),
            "wg1": L1["w"][c].astype(np.float32), "dis1d": dis_dst[0][c],
            "idx2": wrap16(i2), "q2": q2.astype(ml_dtypes.bfloat16),
            "wg2": L2["w"][c].astype(ml_dtypes.bfloat16), "dis2d": dis_dst[1][c],
            "idx3": wrap16(i3), "q3": q3.astype(ml_dtypes.bfloat16),
            "wg3": L3["w"][c].astype(ml_dtypes.bfloat16), "dis3d": dis_dst[2][c],
            "idx4": wrap16(i4), "q4": q4.astype(ml_dtypes.bfloat16),
            "wg4": L4["w"][c].astype(np.float32), "dis4d": dis_dst[3][c],
            "w4blk": blkdiag(W4p, G3),
        }
        in_maps.append(m)

    def reassemble(outs):
        A4f = np.concatenate([np.asarray(outs[c]["outbuf"], np.float32)
                              for c in range(NC)], axis=0)
        return np.ascontiguousarray(A4f[frows][:, :3].astype(np.float32))

    return meta, in_maps, reassemble


# ----------------------------------------------------------------------------
# device kernel
# ----------------------------------------------------------------------------

def build_kernel(meta):
    import concourse.bass as bass
    import concourse.mybir as mybir
    from concourse.bacc import Bacc
    from concourse.tile import TileContext
    from concourse import library_config

    f32 = mybir.dt.float32
    bf16 = mybir.dt.bfloat16
    i16 = mybir.dt.int16
    n = meta["n"]
    Ks, Fs = meta["Ks"], meta["Fs"]

    nc = Bacc("TRN2", target_bir_lowering=False, debug=False, num_devices=NC,
              num_swdge_queues=NQ)

    T1_d = nc.dram_tensor("T1", [meta["T1_rows"] * Ks["1"], 4], f32,
                          kind="ExternalInput")
    ident_d = nc.dram_tensor("ident", [P, P], f32, kind="ExternalInput")
    qio_d = nc.dram_tensor("qio", [P, 32], bf16, kind="ExternalInput")
    w1blk_d = nc.dram_tensor("w1blk", [16 * 4, 16 * 32], f32, kind="ExternalInput")
    w2blk_d = nc.dram_tensor("w2blk", [2 * 32, 2 * 64], f32, kind="ExternalInput")
    w3blk_d = nc.dram_tensor("w3blk", [2 * 64, 2 * 32], f32, kind="ExternalInput")
    w4blk_d = nc.dram_tensor("w4blk", [4 * 32, 4 * 4], f32, kind="ExternalInput")

    L1m, L2m, L3m, L4m = meta["L1"], meta["L2"], meta["L3"], meta["L4"]

    def grid_in(name, C, dt, scale=8):
        # idx grids are [P, C*128/16] int16; q/w grids [P, C]
        return nc.dram_tensor(name, [P, C * scale], dt, kind="ExternalInput")

    idx1 = grid_in("idx1", L1m["C"], i16); q1 = grid_in("q1", L1m["C"], bf16, 1)
    wg1 = grid_in("wg1", L1m["C"], f32, 1)
    dis1d = grid_in("dis1d", L1m["shardP"] // P, f32, 1)
    idx2 = grid_in("idx2", L2m["C"], i16); q2 = grid_in("q2", L2m["C"], bf16, 1)
    wg2 = grid_in("wg2", L2m["C"], bf16, 1)
    dis2d = grid_in("dis2d", L2m["shardP"] // P, f32, 1)
    idx3 = grid_in("idx3", L3m["C"], i16); q3 = grid_in("q3", L3m["C"], bf16, 1)
    wg3 = grid_in("wg3", L3m["C"], bf16, 1)
    dis3d = grid_in("dis3d", L3m["shardP"] // P, f32, 1)
    idx4 = grid_in("idx4", L4m["C"], i16); q4 = grid_in("q4", L4m["C"], bf16, 1)
    wg4 = grid_in("wg4", L4m["C"], f32, 1)
    dis4d = grid_in("dis4d", L4m["shardP"] // P, f32, 1)

    outbuf = nc.dram_tensor("outbuf", [L4m["shardP"], 4], bf16,
                            kind="ExternalOutput")
    rg = [list(range(NC))]

    qctr = [0]

    def next_q():
        q = qctr[0] % NQ
        qctr[0] += 1
        return q

    with TileContext(nc) as tc:
        nc.gpsimd.load_library(library_config.mlp)
        with (
            tc.tile_pool(name="dramp", bufs=1, space="DRAM") as dramp,
            tc.tile_pool(name="consts", bufs=1) as constp,
            tc.tile_pool(name="idxp", bufs=5) as idxp,
            tc.tile_pool(name="gath", bufs=5) as gathp,
            tc.tile_pool(name="work", bufs=3) as workp,
            tc.tile_pool(name="outp", bufs=2) as outp,
            tc.tile_pool(name="psumT", bufs=2, space="PSUM") as psumTp,
            tc.tile_pool(name="psumM", bufs=2, space="PSUM") as psumMp,
        ):
            # persistent DRAM intermediates (bf16 tables)
            A1sh = dramp.tile([L1m["shardP"], 32], bf16)
            T2f = dramp.tile([NC * L1m["shardP"], 32], bf16, addr_space="Shared")
            B2sh = dramp.tile([L2m["shardP"], 32], bf16)
            T3f = dramp.tile([NC * L2m["shardP"], 32], bf16, addr_space="Shared")
            B3sh = dramp.tile([L3m["shardP"], 4], f32)
            T4f = dramp.tile([NC * L3m["shardP"], 4], f32, addr_space="Shared")

            # constants
            ident = constp.tile([P, P], f32)
            nc.sync.dma_start(out=ident[:], in_=ident_d[:, :])
            qio16 = constp.tile([P, 32], bf16)
            nc.sync.dma_start(out=qio16[:], in_=qio_d[:, :])
            w1b = constp.tile([64, 16 * 32], f32)
            nc.sync.dma_start(out=w1b[:], in_=w1blk_d[:, :])
            w2b = constp.tile([64, 2 * 64], f32)
            nc.sync.dma_start(out=w2b[:], in_=w2blk_d[:, :])
            w3b = constp.tile([2 * 64, 2 * 32], f32)
            nc.sync.dma_start(out=w3b[:], in_=w3blk_d[:, :])
            w4b = constp.tile([4 * 32, 4 * 4], f32)
            nc.sync.dma_start(out=w4b[:], in_=w4blk_d[:, :])

            def agg_phase(lm, K, F, idx_d, q_d, w_d, disd_d, table_view,
                          wblk, out_dram, out_f, second=None, tag="",
                          tdt=bf16):
                """One GCN layer aggregation over the padded-CSR grid.

                table_view: DRAM AP [rows, K*F] bf16.
                wblk: None -> identity collapse (gathered feats are final);
                      else (tile, Gfi, Gfo) block-diag matmul after reduce.
                second: optional (w3b-style tile, fi, fo) fused second
                      matmul producing out rows (for B2).
                """
                t0 = 0
                col = 0
                for gi, (Gg, kt) in enumerate(lm["groups"]):
                    ncols = Gg * kt
                    nslots = ncols * P
                    # gathered tile [P, ncols, K*F] bf16 (slot i -> i%128,i//128)
                    gt = gathp.tile([P, ncols * K * F], tdt, tag="g",
                                    name=f"g{tag}_{gi}")
                    # gather in chunks of MAXIDX slots (=MAXIDX/128 cols)
                    ccols = MAXIDX // P
                    for c0 in range(0, ncols, ccols):
                        cw = min(ccols, ncols - c0)
                        idxt = idxp.tile([P, cw * P // 16], i16, tag="i",
                                         name=f"i{tag}_{gi}_{c0}")
                        nc.sync.dma_start(
                            out=idxt[:],
                            in_=idx_d[:, (col + c0) * 8:(col + c0 + cw) * 8])
                        nc.gpsimd.dma_gather(
                            out_ap=gt[:, c0 * K * F:(c0 + cw) * K * F]
                                .rearrange("p (m e) -> p m e", e=K * F),
                            in_ap=table_view,
                            idxs_ap=idxt[:],
                            num_idxs=cw * P, num_idxs_reg=cw * P,
                            elem_size=K * F,
                            single_packet=False, queue_num=next_q())
                    # subpos mask: ind[p, c, K] = (q[p,c] == qio[K])
                    qt = idxp.tile([P, ncols], bf16, tag="q",
                                   name=f"q{tag}_{gi}")
                    nc.sync.dma_start(out=qt[:], in_=q_d[:, col:col + ncols])
                    wt = idxp.tile([P, ncols], tdt, tag="w",
                                   name=f"w{tag}_{gi}")
                    nc.sync.dma_start(out=wt[:], in_=w_d[:, col:col + ncols])
                    ind = workp.tile([P, ncols * K], tdt, tag="n",
                                     name=f"n{tag}_{gi}")
                    nc.vector.tensor_tensor(
                        out=ind[:].rearrange("p (c k) -> p c k", k=K),
                        in0=qt[:].to_broadcast([P, ncols, K]),
                        in1=qio16[:, :K].rearrange("p (o k) -> p o k", o=1)
                            .to_broadcast([P, ncols, K]),
                        op=mybir.AluOpType.is_equal)
                    # fold w: mw = ind * w
                    nc.vector.tensor_tensor(
                        out=ind[:].rearrange("p (c k) -> p c k", k=K),
                        in0=ind[:].rearrange("p (c k) -> p c k", k=K),
                        in1=wt[:].to_broadcast([P, ncols, K]),
                        op=mybir.AluOpType.mult)
                    # apply to gathered rows
                    nc.vector.tensor_tensor(
                        out=gt[:].rearrange("p (c k f) -> p c k f", k=K, f=F),
                        in0=gt[:].rearrange("p (c k f) -> p c k f", k=K, f=F),
                        in1=ind[:].rearrange("p (c k) -> p c k", k=K)
                            .to_broadcast([P, ncols, K, F]),
                        op=mybir.AluOpType.mult)
                    # segment reduce over (kt*K) keeping F
                    S = workp.tile([P, Gg * F], f32, tag="S",
                                   name=f"S{tag}_{gi}")
                    nc.vector.tensor_reduce(
                        out=S[:].rearrange("p (g f) -> p g f", f=F),
                        in_=gt[:].rearrange("p (g x f) -> p g f x",
                                            g=Gg, x=kt * K),
                        axis=mybir.AxisListType.X, op=mybir.AluOpType.add)
                    # dis_dst scale
                    dcol = idxp.tile([P, Gg], f32, tag="d",
                                     name=f"d{tag}_{gi}")
                    nc.sync.dma_start(out=dcol[:], in_=disd_d[:, t0:t0 + Gg])
                    nc.vector.tensor_tensor(
                        out=S[:].rearrange("p (g f) -> p g f", f=F),
                        in0=S[:].rearrange("p (g f) -> p g f", f=F),
                        in1=dcol[:].to_broadcast([P, Gg, F]),
                        op=mybir.AluOpType.mult)
                    if wblk is not None:
                        wtile, gfi, gfo = wblk
                        pT = psumTp.tile([Gg * gfi, P], f32, tag="pT",
                                         name=f"pT{tag}_{gi}")
                        nc.tensor.transpose(out=pT[:], in_=S[:],
                                            identity=ident[:])
                        ST = workp.tile([Gg * gfi, P], f32, tag="ST",
                                        name=f"ST{tag}_{gi}")
                        nc.scalar.copy(out=ST[:], in_=pT[:])
                        pM = psumMp.tile([P, Gg * gfo], f32, tag="pM",
                                         name=f"pM{tag}_{gi}")
                        nc.tensor.matmul(out=pM[:], lhsT=ST[:],
                                         rhs=wtile[:Gg * gfi, :Gg * gfo],
                                         start=True, stop=True)
                        src_ap = pM
                        fo = gfo
                    else:
                        src_ap = S
                        fo = F
                    at = outp.tile([P, Gg * fo], f32, tag="A",
                                   name=f"A{tag}_{gi}")
                    nc.scalar.activation(
                        out=at[:], in_=src_ap[:],
                        func=mybir.ActivationFunctionType.Relu)
                    if second is None:
                        ab = outp.tile([P, Gg * fo], bf16, tag="Ab",
                                       name=f"Ab{tag}_{gi}")
                        nc.vector.tensor_copy(out=ab[:], in_=at[:])
                        nc.sync.dma_start(
                            out=out_dram[t0 * P:(t0 + Gg) * P, :].rearrange(
                                "(g p) f -> p g f", p=P),
                            in_=ab[:])
                    else:
                        stile, sfi, sfo = second
                        pT2 = psumTp.tile([Gg * sfi, P], f32, tag="pT",
                                          name=f"pT2{tag}_{gi}")
                        nc.tensor.transpose(out=pT2[:], in_=at[:],
                                            identity=ident[:])
                        ST2 = workp.tile([Gg * sfi, P], f32, tag="ST2",
                                         name=f"ST2{tag}_{gi}")
                        nc.scalar.copy(out=ST2[:], in_=pT2[:])
                        pM2 = psumMp.tile([P, Gg * sfo], f32, tag="pM",
                                          name=f"pM2{tag}_{gi}")
                        nc.tensor.matmul(out=pM2[:], lhsT=ST2[:],
                                         rhs=stile[:Gg * sfi, :Gg * sfo],
                                         start=True, stop=True)
                        ab = outp.tile([P, Gg * sfo],
                                       f32 if tag == "3" else bf16, tag="Ab2",
                                       name=f"Ab2{tag}_{gi}")
                        nc.vector.tensor_copy(out=ab[:], in_=pM2[:])
                        nc.sync.dma_start(
                            out=out_dram[t0 * P:(t0 + Gg) * P, :].rearrange(
                                "(g p) f -> p g f", p=P),
                            in_=ab[:])
                    t0 += Gg
                    col += ncols

            # ---------------- L1 ----------------
            agg_phase(L1m, Ks["1"], Fs["1"], idx1, q1, wg1, dis1d,
                      T1_d[:, :].rearrange("(r k) f -> r (k f)", k=Ks["1"]),
                      (w1b, 4, 32), A1sh[:], 32, tag="1", tdt=f32)
            nc.gpsimd.collective_compute(
                "AllGather", mybir.AluOpType.bypass, replica_groups=rg,
                ins=[A1sh[:]], outs=[T2f[:]])
            # ---------------- L2 (fused B2 = relu(...)@W3) ----------------
            agg_phase(L2m, Ks["2"], Fs["2"], idx2, q2, wg2, dis2d,
                      T2f[:].rearrange("(r k) f -> r (k f)", k=Ks["2"]),
                      (w2b, 32, 64), B2sh[:], 32, second=(w3b, 64, 32),
                      tag="2")
            nc.gpsimd.collective_compute(
                "AllGather", mybir.AluOpType.bypass, replica_groups=rg,
                ins=[B2sh[:]], outs=[T3f[:]])
            # ---------------- L3 (identity collapse; B3 = relu@W4) --------
            agg_phase(L3m, Ks["3"], Fs["3"], idx3, q3, wg3, dis3d,
                      T3f[:].rearrange("(r k) f -> r (k f)", k=Ks["3"]),
                      None, B3sh[:], 4, second=(w4b, 32, 4), tag="3")
            nc.gpsimd.collective_compute(
                "AllGather", mybir.AluOpType.bypass, replica_groups=rg,
                ins=[B3sh[:]], outs=[T4f[:]])
            # ---------------- L4 ----------------
            agg_phase(L4m, Ks["4"], Fs["4"], idx4, q4, wg4, dis4d,
                      T4f[:].rearrange("(r k) f -> r (k f)", k=Ks["4"]),
                      None, outbuf[:, :], 4, tag="4", tdt=f32)

    nc.finalize()
    return nc


# ----------------------------------------------------------------------------
# PJRT runner (persistent compiled callable, device-resident inputs)
# ----------------------------------------------------------------------------
import numpy as np, time
import jax
import jax.numpy as jnp
from jax.sharding import Mesh, PartitionSpec, NamedSharding
from jax.experimental.shard_map import shard_map
from concourse import mybir
from concourse.bass2jax import _bass_exec_p, partition_id_tensor, install_neuronx_cc_hook


def make_runner(nc, n_cores=8):
    install_neuronx_cc_hook()
    partition_name = nc.partition_id_tensor.name if nc.partition_id_tensor else None
    in_names, out_names, out_avals = [], [], []
    for alloc in nc.m.functions[0].allocations:
        if not isinstance(alloc, mybir.MemoryLocationSet):
            continue
        name = alloc.memorylocations[0].name
        if alloc.kind == "ExternalInput":
            if name != partition_name:
                in_names.append(name)
        elif alloc.kind == "ExternalOutput":
            out_names.append(name)
            out_avals.append(jax.core.ShapedArray(
                tuple(alloc.tensor_shape), mybir.dt.np(alloc.dtype)))
    n_params = len(in_names)
    all_in_names = list(in_names) + list(out_names)
    if partition_name is not None:
        all_in_names.append(partition_name)

    def _body(*args):
        operands = list(args)
        if partition_name is not None:
            operands.append(partition_id_tensor())
        outs = _bass_exec_p.bind(
            *operands,
            out_avals=tuple(out_avals), in_names=tuple(all_in_names),
            out_names=tuple(out_names), lowering_input_output_aliases=(),
            sim_require_finite=False, sim_require_nnan=False, nc=nc)
        return tuple(outs)

    devices = jax.devices()[:n_cores]
    mesh = Mesh(np.asarray(devices), ("core",))
    n_outs = len(out_avals)
    in_specs = (PartitionSpec("core"),) * (n_params + n_outs)
    out_specs = (PartitionSpec("core"),) * len(out_names)
    sharded = jax.jit(shard_map(_body, mesh=mesh, in_specs=in_specs,
                                out_specs=out_specs, check_rep=False),
                      keep_unused=True)
    sharding = NamedSharding(mesh, PartitionSpec("core"))

    state = {}

    def prepare(in_maps):
        per_core = [[np.asarray(m[name]) for name in in_names] for m in in_maps]
        concat_in = [np.concatenate([per_core[c][i] for c in range(n_cores)], axis=0)
                     for i in range(n_params)]
        zeros = [np.zeros((n_cores * av.shape[0], *av.shape[1:]), av.dtype)
                 for av in out_avals]
        state["dev_in"] = [jax.device_put(a, sharding) for a in concat_in + zeros]
        jax.block_until_ready(state["dev_in"])

    def run():
        out = jax.block_until_ready(sharded(*state["dev_in"]))
        return out

    def fetch(out_arrs):
        return [
            {name: np.asarray(out_arrs[i]).reshape(n_cores, *out_avals[i].shape)[c]
             for i, name in enumerate(out_names)}
            for c in range(n_cores)
        ]

    return prepare, run, fetch



_CACHE = {}


def kernel(**inputs):
    for b in ("b1", "b2", "b3", "b4"):
        if b in inputs:
            assert not np.asarray(inputs[b]).any()
    meta, in_maps, reassemble = prep(inputs)
    if "k" not in _CACHE:
        nc = build_kernel(meta)
        _CACHE["k"] = make_runner(nc)
    prepare, run, fetch = _CACHE["k"]
    prepare(in_maps)
    outs = fetch(run())
    return reassemble(outs).astype(np.float32)


# revision 21
# speedup vs baseline: 1.7358x; 1.7358x over previous
"""Trainium2 Bass kernel v2 for the 4-layer GCN + mesh-unpool network,
8 NeuronCores, dst-sharded graph parallelism.

vs v1 (baseline): per-column indirect DMA gathers (994ns Q7 emission per
128 rows -> ~14ms) are replaced by bulk InstDMAGatherAnt gathers
(~8k rows per instruction, round-robin over 4 SWDGE queues), with bf16
tables packed K nodes per 256B+ row (pure reinterpretation of the
row-major [V, F] table).  W3/W4 are pushed through the aggregation
(linearity), eliminating both AllToAll exchanges and the fetch/build
phases; unpools become host-side index composition.

Per layer l (dst shard on each core): padded-CSR slot grids over
in-degree-sorted nodes; slots gathered from the bf16 table (row idx//K,
sub-node idx%K selected by a DVE mask), weighted by dis_l[src] (and
killed for pad slots), segment-reduced, scaled by dis_l[dst], matmul'd
(W1/W2; identity collapse for L3/L4 which gather pre-multiplied
tables), relu'd, stored bf16, AllGathered into the next table.
"""
import sys
sys.path.insert(0, "/opt/trn_rl_repo")

import numpy as np
import ml_dtypes

NC = 8
P = 128
NQ = 4            # SWDGE queues
MAXIDX = 8192     # idx per dma_gather instruction (HW-validated)


def pad_to(x, m):
    return (x + m - 1) // m * m


# ----------------------------------------------------------------------------
# host-side planning
# ----------------------------------------------------------------------------

def make_dis(edge_index, n):
    deg = np.bincount(edge_index[1], minlength=n).astype(np.float64) + 1.0
    return (1.0 / np.sqrt(deg)).astype(np.float32)


def plan_agg(dst_old, slot_row, n, self_row, slot_weight, self_weight, G,
             col_budget=64):
    """Padded-CSR plan (degree-sorted, program-unified across cores).

    Returns per-core idx [P, Ctot] int32 (table row ids), w [P, Ctot]
    fp32, groups [(Gg, kt)], sigma (old id -> global new row), dis-dst
    columns come separately.
    """
    shard = n // NC
    shardP = pad_to(shard, P)
    ntiles = shardP // P
    c_of = dst_old // shard

    percore = []
    for c in range(NC):
        m = c_of == c
        dl = dst_old[m] - c * shard
        deg = np.bincount(dl, minlength=shard)
        perm = np.argsort(-deg, kind="stable")
        inv = np.empty_like(perm); inv[perm] = np.arange(shard)
        percore.append((m, dl, deg, perm, inv))

    def span_kt(lo_t, Gg):
        lo, hi = lo_t * P, min((lo_t + Gg) * P, shard)
        kt = 0
        for c in range(NC):
            deg_new = percore[c][2][percore[c][3]]
            if hi > lo:
                kt = max(kt, int(deg_new[lo:hi].max()))
        return kt + 1

    groups = []
    t = 0
    while t < ntiles:
        Gg = min(G, ntiles - t)
        kt = span_kt(t, Gg)
        while Gg > 1 and Gg * kt > col_budget:
            Gg = max(1, min(Gg - 1, col_budget // kt))
            kt = span_kt(t, Gg)
        groups.append((Gg, kt))
        t += Gg

    colbase = np.zeros(ntiles + 1, np.int64)
    kts_tile = []
    for (Gg, kt) in groups:
        kts_tile += [kt] * Gg
    for t in range(ntiles):
        colbase[t + 1] = colbase[t] + kts_tile[t]
    Ctot = int(colbase[-1])

    out = {"groups": groups, "idx": [], "w": [], "perm": [],
           "shardP": shardP, "Ctot": Ctot}
    sigma = np.empty(n, np.int64)
    for c in range(NC):
        m, dl, deg, perm, inv = percore[c]
        sigma[c * shard:(c + 1) * shard] = c * shardP + inv
        sr = slot_row[m]
        sw = slot_weight[m]
        nd = inv[dl]
        order = np.argsort(nd, kind="stable")
        nd_s = nd[order]; sr_s = sr[order]
        deg_new = deg[perm]
        starts = np.zeros(shard + 1, np.int64)
        np.cumsum(deg_new, out=starts[1:])
        idx = np.zeros((P, Ctot), np.int64)
        w = np.zeros((P, Ctot), np.float32)
        r = np.arange(len(nd_s)) - starts[nd_s]
        pp_ = nd_s % P
        tt_ = nd_s // P
        cols = colbase[tt_] + r
        idx[pp_, cols] = sr_s
        w[pp_, cols] = sw[order]
        v = np.arange(shard)
        scols = colbase[v // P] + deg_new[v]
        idx[v % P, scols] = self_row[perm + c * shard]
        w[v % P, scols] = self_weight[perm + c * shard]
        out["idx"].append(idx)
        out["w"].append(w)
        out["perm"].append(perm)
    out["sigma"] = sigma
    return out


def tile_cols(vec_percore, shardP):
    outs = []
    for v in vec_percore:
        a = np.zeros(shardP, np.float32)
        a[:len(v)] = v
        outs.append(a.reshape(shardP // P, P).T.copy())
    return outs


def wrap16(idx_cols):
    """[P, C] column-major slot grid -> dma_gather idx layout.

    Slot i (= c*128 + p) must sit at (partition i%16, col i//16),
    replicated across the 8 16-partition groups. Returns [P, C*8] int16.
    """
    Pp, C = idx_cols.shape
    flat = idx_cols.T.reshape(-1)              # slot i order
    n = flat.shape[0]
    w = flat.reshape(n // 16, 16).T            # [16, n/16]
    return np.tile(w, (8, 1)).astype(np.int16) # [128, n/16]


def split_kq(idx, K):
    return (idx // K).astype(np.int64), (idx % K).astype(np.float32)


def prep(inputs, G1=16, G2=4, G3=4, G4=16, GF=8):
    x = np.asarray(inputs["x"], np.float32)
    W = [np.asarray(inputs[f"W{i}"], np.float32) for i in (1, 2, 3, 4)]
    ei = [np.asarray(inputs[f"edge_index{i}"]).astype(np.int64) for i in range(4)]
    u = [np.asarray(inputs[f"unpool{i}"]).astype(np.int64) for i in (1, 2, 3, 4)]
    n = [x.shape[0], len(u[0]), len(u[1]), len(u[2])]
    nout = len(u[3])
    dis = [make_dis(ei[l], n[l]) for l in range(4)]

    # L1: table rows = original x ids (dis1 folded into table)
    L1 = plan_agg(ei[0][1], ei[0][0], n[0], self_row=np.arange(n[0]), G=G1,
                  slot_weight=np.ones(ei[0].shape[1], np.float32),
                  self_weight=np.ones(n[0], np.float32), col_budget=128)
    s1 = L1["sigma"]
    # L2: rows in T2 (=A1, global new order), composed via u1
    L2 = plan_agg(ei[1][1], s1[u[0][ei[1][0]]], n[1], self_row=s1[u[0]], G=G2,
                  slot_weight=dis[1][ei[1][0]], self_weight=dis[1])
    s2 = L2["sigma"]
    # L3: rows in T3 (=B2 = A2@W3), composed via u2
    L3 = plan_agg(ei[2][1], s2[u[1][ei[2][0]]], n[2], self_row=s2[u[1]], G=G3,
                  slot_weight=dis[2][ei[2][0]], self_weight=dis[2])
    s3 = L3["sigma"]
    # L4: rows in T4 (=B3 = A3@W4), composed via u3
    L4 = plan_agg(ei[3][1], s3[u[2][ei[3][0]]], n[3], self_row=s3[u[2]], G=G4,
                  slot_weight=dis[3][ei[3][0]], self_weight=dis[3],
                  col_budget=128)
    s4 = L4["sigma"]

    # dis-dst columns (new-local order per core)
    dis_dst = []
    for l, L in enumerate((L1, L2, L3, L4)):
        sh = n[l] // NC
        dis_dst.append(tile_cols(
            [dis[l][c * sh + L["perm"][c]] for c in range(NC)], L["shardP"]))

    # per-layer packing K (nodes per 256B+ table row, bf16)
    Ks = {"1": 16, "2": 4, "3": 8, "4": 16, "F": 32}
    Fs = {"1": 4, "2": 32, "3": 32, "4": 4, "F": 4}

    # final unpool is applied on the host during unshard/reassembly
    shf = nout // NC
    shfP = pad_to(shf, P)
    CF = shfP // P
    frows = s4[u[3]]

    meta = dict(
        n=n, nout=nout, shf=shf, shfP=shfP, CF=CF, Ks=Ks, Fs=Fs,
        L1=dict(groups=L1["groups"], shardP=L1["shardP"], C=L1["Ctot"],
                fout=32, wmat="w1", relu=True),
        L2=dict(groups=L2["groups"], shardP=L2["shardP"], C=L2["Ctot"],
                fout=64, wmat="w2", relu=True),
        L3=dict(groups=L3["groups"], shardP=L3["shardP"], C=L3["Ctot"],
                fout=32, wmat=None, relu=True),
        L4=dict(groups=L4["groups"], shardP=L4["shardP"], C=L4["Ctot"],
                fout=4, wmat=None, relu=True),
        T1_rows=pad_to(n[0], Ks["1"]) // Ks["1"],
        T2_rows=NC * L1["shardP"] // Ks["2"],
        T3_rows=NC * L2["shardP"] // Ks["3"],
        T4_rows=NC * L3["shardP"] // Ks["4"],
        T5_rows=NC * L4["shardP"] // Ks["F"],
    )

    # ---- per-core inputs ----
    # T1: x * dis1, padded to 4 feats, bf16, row-major [V,4] (viewed packed)
    V1 = pad_to(n[0], Ks["1"])
    T1 = np.zeros((V1, 4), np.float32)
    T1[:n[0], :3] = x * dis[0][:, None]

    W1p = np.zeros((4, 32), np.float32); W1p[:3] = W[0]
    W4p = np.zeros((32, 4), np.float32); W4p[:, :3] = W[3]

    def blkdiag(Wm, G):
        fi, fo = Wm.shape
        B = np.zeros((G * fi, G * fo), np.float32)
        for g in range(G):
            B[g * fi:(g + 1) * fi, g * fo:(g + 1) * fo] = Wm
        return B

    ident = np.eye(P, dtype=np.float32)
    qio = np.tile(np.arange(32, dtype=np.float32)[None, :], (128, 1)).astype(ml_dtypes.bfloat16)

    in_maps = []
    for c in range(NC):
        i1, q1 = split_kq(L1["idx"][c], Ks["1"])
        i2, q2 = split_kq(L2["idx"][c], Ks["2"])
        i3, q3 = split_kq(L3["idx"][c], Ks["3"])
        i4, q4 = split_kq(L4["idx"][c], Ks["4"])
        m = {
            "T1": T1, "ident": ident, "qio": qio,
            "w1blk": blkdiag(W1p, G1), "w2blk": blkdiag(W[1], G2),
            "w3blk": blkdiag(W[2], 2),
            "idx1": wrap16(i1), "q1": q1.astype(ml_dtypes.bfloat16),
            "wg1": L1["w"][c].astype(np.float32), "dis1d": dis_dst[0][c],
            "idx2": wrap16(i2), "q2": q2.astype(ml_dtypes.bfloat16),
            "wg2": L2["w"][c].astype(ml_dtypes.bfloat16), "dis2d": dis_dst[1][c],
            "idx3": wrap16(i3), "q3": q3.astype(ml_dtypes.bfloat16),
            "wg3": L3["w"][c].astype(ml_dtypes.bfloat16), "dis3d": dis_dst[2][c],
            "idx4": wrap16(i4), "q4": q4.astype(ml_dtypes.bfloat16),
            "wg4": L4["w"][c].astype(np.float32), "dis4d": dis_dst[3][c],
            "w4blk": blkdiag(W4p, G3),
        }
        in_maps.append(m)

    def reassemble(outs):
        A4f = np.concatenate([np.asarray(outs[c]["outbuf"], np.float32)
                              for c in range(NC)], axis=0)
        return np.ascontiguousarray(A4f[frows][:, :3].astype(np.float32))

    return meta, in_maps, reassemble


# ----------------------------------------------------------------------------
# device kernel
# ----------------------------------------------------------------------------

def build_kernel(meta):
    import concourse.bass as bass
    import concourse.mybir as mybir
    from concourse.bacc import Bacc
    from concourse.tile import TileContext
    from concourse import library_config

    f32 = mybir.dt.float32
    bf16 = mybir.dt.bfloat16
    i16 = mybir.dt.int16
    n = meta["n"]
    Ks, Fs = meta["Ks"], meta["Fs"]

    nc = Bacc("TRN2", target_bir_lowering=False, debug=False, num_devices=NC,
              num_swdge_queues=NQ)

    T1_d = nc.dram_tensor("T1", [meta["T1_rows"] * Ks["1"], 4], f32,
                          kind="ExternalInput")
    ident_d = nc.dram_tensor("ident", [P, P], f32, kind="ExternalInput")
    qio_d = nc.dram_tensor("qio", [P, 32], bf16, kind="ExternalInput")
    w1blk_d = nc.dram_tensor("w1blk", [16 * 4, 16 * 32], f32, kind="ExternalInput")
    w2blk_d = nc.dram_tensor("w2blk", [4 * 32, 4 * 64], f32, kind="ExternalInput")
    w3blk_d = nc.dram_tensor("w3blk", [2 * 64, 2 * 32], f32, kind="ExternalInput")
    w4blk_d = nc.dram_tensor("w4blk", [4 * 32, 4 * 4], f32, kind="ExternalInput")

    L1m, L2m, L3m, L4m = meta["L1"], meta["L2"], meta["L3"], meta["L4"]

    def grid_in(name, C, dt, scale=8):
        # idx grids are [P, C*128/16] int16; q/w grids [P, C]
        return nc.dram_tensor(name, [P, C * scale], dt, kind="ExternalInput")

    idx1 = grid_in("idx1", L1m["C"], i16); q1 = grid_in("q1", L1m["C"], bf16, 1)
    wg1 = grid_in("wg1", L1m["C"], f32, 1)
    dis1d = grid_in("dis1d", L1m["shardP"] // P, f32, 1)
    idx2 = grid_in("idx2", L2m["C"], i16); q2 = grid_in("q2", L2m["C"], bf16, 1)
    wg2 = grid_in("wg2", L2m["C"], bf16, 1)
    dis2d = grid_in("dis2d", L2m["shardP"] // P, f32, 1)
    idx3 = grid_in("idx3", L3m["C"], i16); q3 = grid_in("q3", L3m["C"], bf16, 1)
    wg3 = grid_in("wg3", L3m["C"], bf16, 1)
    dis3d = grid_in("dis3d", L3m["shardP"] // P, f32, 1)
    idx4 = grid_in("idx4", L4m["C"], i16); q4 = grid_in("q4", L4m["C"], bf16, 1)
    wg4 = grid_in("wg4", L4m["C"], f32, 1)
    dis4d = grid_in("dis4d", L4m["shardP"] // P, f32, 1)

    outbuf = nc.dram_tensor("outbuf", [L4m["shardP"], 4], bf16,
                            kind="ExternalOutput")
    rg = [list(range(NC))]

    qctr = [0]

    def next_q():
        q = qctr[0] % NQ
        qctr[0] += 1
        return q

    with TileContext(nc) as tc:
        nc.gpsimd.load_library(library_config.mlp)
        with (
            tc.tile_pool(name="dramp", bufs=1, space="DRAM") as dramp,
            tc.tile_pool(name="consts", bufs=1) as constp,
            tc.tile_pool(name="idxp", bufs=4) as idxp,
            tc.tile_pool(name="gath", bufs=5) as gathp,
            tc.tile_pool(name="work", bufs=3) as workp,
            tc.tile_pool(name="outp", bufs=2) as outp,
            tc.tile_pool(name="psumT", bufs=2, space="PSUM") as psumTp,
            tc.tile_pool(name="psumM", bufs=2, space="PSUM") as psumMp,
        ):
            # persistent DRAM intermediates (bf16 tables)
            A1sh = dramp.tile([L1m["shardP"], 32], bf16)
            T2f = dramp.tile([NC * L1m["shardP"], 32], bf16, addr_space="Shared")
            B2sh = dramp.tile([L2m["shardP"], 32], bf16)
            T3f = dramp.tile([NC * L2m["shardP"], 32], bf16, addr_space="Shared")
            B3sh = dramp.tile([L3m["shardP"], 4], f32)
            T4f = dramp.tile([NC * L3m["shardP"], 4], f32, addr_space="Shared")

            # constants
            ident = constp.tile([P, P], f32)
            nc.sync.dma_start(out=ident[:], in_=ident_d[:, :])
            qio16 = constp.tile([P, 32], bf16)
            nc.sync.dma_start(out=qio16[:], in_=qio_d[:, :])
            w1b = constp.tile([64, 16 * 32], f32)
            nc.sync.dma_start(out=w1b[:], in_=w1blk_d[:, :])
            w2b = constp.tile([128, 4 * 64], f32)
            nc.sync.dma_start(out=w2b[:], in_=w2blk_d[:, :])
            w3b = constp.tile([2 * 64, 2 * 32], f32)
            nc.sync.dma_start(out=w3b[:], in_=w3blk_d[:, :])
            w4b = constp.tile([4 * 32, 4 * 4], f32)
            nc.sync.dma_start(out=w4b[:], in_=w4blk_d[:, :])

            def agg_phase(lm, K, F, idx_d, q_d, w_d, disd_d, table_view,
                          wblk, out_dram, out_f, second=None, tag="",
                          tdt=bf16):
                """One GCN layer aggregation over the padded-CSR grid.

                table_view: DRAM AP [rows, K*F] bf16.
                wblk: None -> identity collapse (gathered feats are final);
                      else (tile, Gfi, Gfo) block-diag matmul after reduce.
                second: optional (w3b-style tile, fi, fo) fused second
                      matmul producing out rows (for B2).
                """
                t0 = 0
                col = 0
                for gi, (Gg, kt) in enumerate(lm["groups"]):
                    ncols = Gg * kt
                    nslots = ncols * P
                    # gathered tile [P, ncols, K*F] bf16 (slot i -> i%128,i//128)
                    gt = gathp.tile([P, ncols * K * F], tdt, tag="g",
                                    name=f"g{tag}_{gi}")
                    # gather in chunks of MAXIDX slots (=MAXIDX/128 cols)
                    ccols = MAXIDX // P
                    for c0 in range(0, ncols, ccols):
                        cw = min(ccols, ncols - c0)
                        idxt = idxp.tile([P, cw * P // 16], i16, tag="i",
                                         name=f"i{tag}_{gi}_{c0}")
                        nc.sync.dma_start(
                            out=idxt[:],
                            in_=idx_d[:, (col + c0) * 8:(col + c0 + cw) * 8])
                        nc.gpsimd.dma_gather(
                            out_ap=gt[:, c0 * K * F:(c0 + cw) * K * F]
                                .rearrange("p (m e) -> p m e", e=K * F),
                            in_ap=table_view,
                            idxs_ap=idxt[:],
                            num_idxs=cw * P, num_idxs_reg=cw * P,
                            elem_size=K * F,
                            single_packet=False, queue_num=next_q())
                    # subpos mask: ind[p, c, K] = (q[p,c] == qio[K])
                    qt = idxp.tile([P, ncols], bf16, tag="q",
                                   name=f"q{tag}_{gi}")
                    nc.sync.dma_start(out=qt[:], in_=q_d[:, col:col + ncols])
                    wt = idxp.tile([P, ncols], tdt, tag="w",
                                   name=f"w{tag}_{gi}")
                    nc.sync.dma_start(out=wt[:], in_=w_d[:, col:col + ncols])
                    ind = workp.tile([P, ncols * K], tdt, tag="n",
                                     name=f"n{tag}_{gi}")
                    nc.vector.tensor_tensor(
                        out=ind[:].rearrange("p (c k) -> p c k", k=K),
                        in0=qt[:].to_broadcast([P, ncols, K]),
                        in1=qio16[:, :K].rearrange("p (o k) -> p o k", o=1)
                            .to_broadcast([P, ncols, K]),
                        op=mybir.AluOpType.is_equal)
                    # fold w: mw = ind * w
                    nc.vector.tensor_tensor(
                        out=ind[:].rearrange("p (c k) -> p c k", k=K),
                        in0=ind[:].rearrange("p (c k) -> p c k", k=K),
                        in1=wt[:].to_broadcast([P, ncols, K]),
                        op=mybir.AluOpType.mult)
                    # apply to gathered rows
                    nc.vector.tensor_tensor(
                        out=gt[:].rearrange("p (c k f) -> p c k f", k=K, f=F),
                        in0=gt[:].rearrange("p (c k f) -> p c k f", k=K, f=F),
                        in1=ind[:].rearrange("p (c k) -> p c k", k=K)
                            .to_broadcast([P, ncols, K, F]),
                        op=mybir.AluOpType.mult)
                    # segment reduce over (kt*K) keeping F
                    S = workp.tile([P, Gg * F], f32, tag="S",
                                   name=f"S{tag}_{gi}")
                    nc.vector.tensor_reduce(
                        out=S[:].rearrange("p (g f) -> p g f", f=F),
                        in_=gt[:].rearrange("p (g x f) -> p g f x",
                                            g=Gg, x=kt * K),
                        axis=mybir.AxisListType.X, op=mybir.AluOpType.add)
                    # dis_dst scale
                    dcol = idxp.tile([P, Gg], f32, tag="d",
                                     name=f"d{tag}_{gi}")
                    nc.sync.dma_start(out=dcol[:], in_=disd_d[:, t0:t0 + Gg])
                    nc.vector.tensor_tensor(
                        out=S[:].rearrange("p (g f) -> p g f", f=F),
                        in0=S[:].rearrange("p (g f) -> p g f", f=F),
                        in1=dcol[:].to_broadcast([P, Gg, F]),
                        op=mybir.AluOpType.mult)
                    if wblk is not None:
                        wtile, gfi, gfo = wblk
                        pT = psumTp.tile([Gg * gfi, P], f32, tag="pT",
                                         name=f"pT{tag}_{gi}")
                        nc.tensor.transpose(out=pT[:], in_=S[:],
                                            identity=ident[:])
                        ST = workp.tile([Gg * gfi, P], f32, tag="ST",
                                        name=f"ST{tag}_{gi}")
                        nc.scalar.copy(out=ST[:], in_=pT[:])
                        pM = psumMp.tile([P, Gg * gfo], f32, tag="pM",
                                         name=f"pM{tag}_{gi}")
                        nc.tensor.matmul(out=pM[:], lhsT=ST[:],
                                         rhs=wtile[:Gg * gfi, :Gg * gfo],
                                         start=True, stop=True)
                        src_ap = pM
                        fo = gfo
                    else:
                        src_ap = S
                        fo = F
                    at = outp.tile([P, Gg * fo], f32, tag="A",
                                   name=f"A{tag}_{gi}")
                    nc.scalar.activation(
                        out=at[:], in_=src_ap[:],
                        func=mybir.ActivationFunctionType.Relu)
                    if second is None:
                        ab = outp.tile([P, Gg * fo], bf16, tag="Ab",
                                       name=f"Ab{tag}_{gi}")
                        nc.vector.tensor_copy(out=ab[:], in_=at[:])
                        nc.sync.dma_start(
                            out=out_dram[t0 * P:(t0 + Gg) * P, :].rearrange(
                                "(g p) f -> p g f", p=P),
                            in_=ab[:])
                    else:
                        stile, sfi, sfo = second
                        sub = max(1, 128 // sfi)
                        ab = outp.tile([P, Gg * sfo],
                                       f32 if tag == "3" else bf16, tag="Ab2",
                                       name=f"Ab2{tag}_{gi}")
                        for s0 in range(0, Gg, sub):
                            sw = min(sub, Gg - s0)
                            pT2 = psumTp.tile([sw * sfi, P], f32, tag="pT",
                                              name=f"pT2{tag}_{gi}_{s0}")
                            nc.tensor.transpose(
                                out=pT2[:],
                                in_=at[:, s0 * sfi:(s0 + sw) * sfi],
                                identity=ident[:])
                            ST2 = workp.tile([sw * sfi, P], f32, tag="ST2",
                                             name=f"ST2{tag}_{gi}_{s0}")
                            nc.scalar.copy(out=ST2[:], in_=pT2[:])
                            pM2 = psumMp.tile([P, sw * sfo], f32, tag="pM",
                                              name=f"pM2{tag}_{gi}_{s0}")
                            nc.tensor.matmul(out=pM2[:], lhsT=ST2[:],
                                             rhs=stile[:sw * sfi, :sw * sfo],
                                             start=True, stop=True)
                            nc.vector.tensor_copy(
                                out=ab[:, s0 * sfo:(s0 + sw) * sfo],
                                in_=pM2[:])
                        nc.sync.dma_start(
                            out=out_dram[t0 * P:(t0 + Gg) * P, :].rearrange(
                                "(g p) f -> p g f", p=P),
                            in_=ab[:])
                    t0 += Gg
                    col += ncols

            # ---------------- L1 ----------------
            agg_phase(L1m, Ks["1"], Fs["1"], idx1, q1, wg1, dis1d,
                      T1_d[:, :].rearrange("(r k) f -> r (k f)", k=Ks["1"]),
                      (w1b, 4, 32), A1sh[:], 32, tag="1", tdt=f32)
            nc.gpsimd.collective_compute(
                "AllGather", mybir.AluOpType.bypass, replica_groups=rg,
                ins=[A1sh[:]], outs=[T2f[:]])
            # ---------------- L2 (fused B2 = relu(...)@W3) ----------------
            agg_phase(L2m, Ks["2"], Fs["2"], idx2, q2, wg2, dis2d,
                      T2f[:].rearrange("(r k) f -> r (k f)", k=Ks["2"]),
                      (w2b, 32, 64), B2sh[:], 32, second=(w3b, 64, 32),
                      tag="2")
            nc.gpsimd.collective_compute(
                "AllGather", mybir.AluOpType.bypass, replica_groups=rg,
                ins=[B2sh[:]], outs=[T3f[:]])
            # ---------------- L3 (identity collapse; B3 = relu@W4) --------
            agg_phase(L3m, Ks["3"], Fs["3"], idx3, q3, wg3, dis3d,
                      T3f[:].rearrange("(r k) f -> r (k f)", k=Ks["3"]),
                      None, B3sh[:], 4, second=(w4b, 32, 4), tag="3")
            nc.gpsimd.collective_compute(
                "AllGather", mybir.AluOpType.bypass, replica_groups=rg,
                ins=[B3sh[:]], outs=[T4f[:]])
            # ---------------- L4 ----------------
            agg_phase(L4m, Ks["4"], Fs["4"], idx4, q4, wg4, dis4d,
                      T4f[:].rearrange("(r k) f -> r (k f)", k=Ks["4"]),
                      None, outbuf[:, :], 4, tag="4", tdt=f32)

    nc.finalize()
    return nc


# ----------------------------------------------------------------------------
# PJRT runner (persistent compiled callable, device-resident inputs)
# ----------------------------------------------------------------------------
import numpy as np, time
import jax
import jax.numpy as jnp
from jax.sharding import Mesh, PartitionSpec, NamedSharding
from jax.experimental.shard_map import shard_map
from concourse import mybir
from concourse.bass2jax import _bass_exec_p, partition_id_tensor, install_neuronx_cc_hook


def make_runner(nc, n_cores=8):
    install_neuronx_cc_hook()
    partition_name = nc.partition_id_tensor.name if nc.partition_id_tensor else None
    in_names, out_names, out_avals = [], [], []
    for alloc in nc.m.functions[0].allocations:
        if not isinstance(alloc, mybir.MemoryLocationSet):
            continue
        name = alloc.memorylocations[0].name
        if alloc.kind == "ExternalInput":
            if name != partition_name:
                in_names.append(name)
        elif alloc.kind == "ExternalOutput":
            out_names.append(name)
            out_avals.append(jax.core.ShapedArray(
                tuple(alloc.tensor_shape), mybir.dt.np(alloc.dtype)))
    n_params = len(in_names)
    all_in_names = list(in_names) + list(out_names)
    if partition_name is not None:
        all_in_names.append(partition_name)

    def _body(*args):
        operands = list(args)
        if partition_name is not None:
            operands.append(partition_id_tensor())
        outs = _bass_exec_p.bind(
            *operands,
            out_avals=tuple(out_avals), in_names=tuple(all_in_names),
            out_names=tuple(out_names), lowering_input_output_aliases=(),
            sim_require_finite=False, sim_require_nnan=False, nc=nc)
        return tuple(outs)

    devices = jax.devices()[:n_cores]
    mesh = Mesh(np.asarray(devices), ("core",))
    n_outs = len(out_avals)
    in_specs = (PartitionSpec("core"),) * (n_params + n_outs)
    out_specs = (PartitionSpec("core"),) * len(out_names)
    sharded = jax.jit(shard_map(_body, mesh=mesh, in_specs=in_specs,
                                out_specs=out_specs, check_rep=False),
                      keep_unused=True)
    sharding = NamedSharding(mesh, PartitionSpec("core"))

    state = {}

    def prepare(in_maps):
        per_core = [[np.asarray(m[name]) for name in in_names] for m in in_maps]
        concat_in = [np.concatenate([per_core[c][i] for c in range(n_cores)], axis=0)
                     for i in range(n_params)]
        zeros = [np.zeros((n_cores * av.shape[0], *av.shape[1:]), av.dtype)
                 for av in out_avals]
        state["dev_in"] = [jax.device_put(a, sharding) for a in concat_in + zeros]
        jax.block_until_ready(state["dev_in"])

    def run():
        out = jax.block_until_ready(sharded(*state["dev_in"]))
        return out

    def fetch(out_arrs):
        return [
            {name: np.asarray(out_arrs[i]).reshape(n_cores, *out_avals[i].shape)[c]
             for i, name in enumerate(out_names)}
            for c in range(n_cores)
        ]

    return prepare, run, fetch



_CACHE = {}


def kernel(**inputs):
    for b in ("b1", "b2", "b3", "b4"):
        if b in inputs:
            assert not np.asarray(inputs[b]).any()
    meta, in_maps, reassemble = prep(inputs)
    if "k" not in _CACHE:
        nc = build_kernel(meta)
        _CACHE["k"] = make_runner(nc)
    prepare, run, fetch = _CACHE["k"]
    prepare(in_maps)
    outs = fetch(run())
    return reassemble(outs).astype(np.float32)


# revision 22
# speedup vs baseline: 2.3712x; 1.3661x over previous
"""Trainium2 Bass kernel v2 for the 4-layer GCN + mesh-unpool network,
8 NeuronCores, dst-sharded graph parallelism.

vs v1 (baseline): per-column indirect DMA gathers (994ns Q7 emission per
128 rows -> ~14ms) are replaced by bulk InstDMAGatherAnt gathers
(~8k rows per instruction, round-robin over 4 SWDGE queues), with bf16
tables packed K nodes per 256B+ row (pure reinterpretation of the
row-major [V, F] table).  W3/W4 are pushed through the aggregation
(linearity), eliminating both AllToAll exchanges and the fetch/build
phases; unpools become host-side index composition.

Per layer l (dst shard on each core): padded-CSR slot grids over
in-degree-sorted nodes; slots gathered from the bf16 table (row idx//K,
sub-node idx%K selected by a DVE mask), weighted by dis_l[src] (and
killed for pad slots), segment-reduced, scaled by dis_l[dst], matmul'd
(W1/W2; identity collapse for L3/L4 which gather pre-multiplied
tables), relu'd, stored bf16, AllGathered into the next table.
"""
import sys
sys.path.insert(0, "/opt/trn_rl_repo")

import numpy as np
import ml_dtypes

NC = 8
P = 128
NQ = 4            # SWDGE queues
MAXIDX = 8192     # idx per dma_gather instruction (HW-validated)


def pad_to(x, m):
    return (x + m - 1) // m * m


# ----------------------------------------------------------------------------
# host-side planning
# ----------------------------------------------------------------------------

def make_dis(edge_index, n):
    deg = np.bincount(edge_index[1], minlength=n).astype(np.float64) + 1.0
    return (1.0 / np.sqrt(deg)).astype(np.float32)


def plan_agg(dst_old, slot_row, n, self_row, slot_weight, self_weight, G,
             col_budget=64):
    """Padded-CSR plan (degree-sorted, program-unified across cores).

    Returns per-core idx [P, Ctot] int32 (table row ids), w [P, Ctot]
    fp32, groups [(Gg, kt)], sigma (old id -> global new row), dis-dst
    columns come separately.
    """
    shard = n // NC
    shardP = pad_to(shard, P)
    ntiles = shardP // P
    c_of = dst_old // shard

    percore = []
    for c in range(NC):
        m = c_of == c
        dl = dst_old[m] - c * shard
        deg = np.bincount(dl, minlength=shard)
        perm = np.argsort(-deg, kind="stable")
        inv = np.empty_like(perm); inv[perm] = np.arange(shard)
        percore.append((m, dl, deg, perm, inv))

    def span_kt(lo_t, Gg):
        lo, hi = lo_t * P, min((lo_t + Gg) * P, shard)
        kt = 0
        for c in range(NC):
            deg_new = percore[c][2][percore[c][3]]
            if hi > lo:
                kt = max(kt, int(deg_new[lo:hi].max()))
        return kt + 1

    groups = []
    t = 0
    while t < ntiles:
        Gg = min(G, ntiles - t)
        kt = span_kt(t, Gg)
        while Gg > 1 and Gg * kt > col_budget:
            Gg = max(1, min(Gg - 1, col_budget // kt))
            kt = span_kt(t, Gg)
        groups.append((Gg, kt))
        t += Gg

    colbase = np.zeros(ntiles + 1, np.int64)
    kts_tile = []
    for (Gg, kt) in groups:
        kts_tile += [kt] * Gg
    for t in range(ntiles):
        colbase[t + 1] = colbase[t] + kts_tile[t]
    Ctot = int(colbase[-1])

    out = {"groups": groups, "idx": [], "w": [], "perm": [],
           "shardP": shardP, "Ctot": Ctot}
    sigma = np.empty(n, np.int64)
    for c in range(NC):
        m, dl, deg, perm, inv = percore[c]
        sigma[c * shard:(c + 1) * shard] = c * shardP + inv
        sr = slot_row[m]
        sw = slot_weight[m]
        nd = inv[dl]
        order = np.argsort(nd, kind="stable")
        nd_s = nd[order]; sr_s = sr[order]
        deg_new = deg[perm]
        starts = np.zeros(shard + 1, np.int64)
        np.cumsum(deg_new, out=starts[1:])
        idx = np.zeros((P, Ctot), np.int64)
        w = np.zeros((P, Ctot), np.float32)
        r = np.arange(len(nd_s)) - starts[nd_s]
        pp_ = nd_s % P
        tt_ = nd_s // P
        cols = colbase[tt_] + r
        idx[pp_, cols] = sr_s
        w[pp_, cols] = sw[order]
        v = np.arange(shard)
        scols = colbase[v // P] + deg_new[v]
        idx[v % P, scols] = self_row[perm + c * shard]
        w[v % P, scols] = self_weight[perm + c * shard]
        out["idx"].append(idx)
        out["w"].append(w)
        out["perm"].append(perm)
    out["sigma"] = sigma
    return out


def tile_cols(vec_percore, shardP):
    outs = []
    for v in vec_percore:
        a = np.zeros(shardP, np.float32)
        a[:len(v)] = v
        outs.append(a.reshape(shardP // P, P).T.copy())
    return outs


def wrap16(idx_cols):
    """[P, C] column-major slot grid -> dma_gather idx layout.

    Slot i (= c*128 + p) must sit at (partition i%16, col i//16),
    replicated across the 8 16-partition groups. Returns [P, C*8] int16.
    """
    Pp, C = idx_cols.shape
    flat = idx_cols.T.reshape(-1)              # slot i order
    n = flat.shape[0]
    w = flat.reshape(n // 16, 16).T            # [16, n/16]
    return np.tile(w, (8, 1)).astype(np.int16) # [128, n/16]


def split_kq(idx, K):
    return (idx // K).astype(np.int64), (idx % K).astype(np.float32)


def prep(inputs, G1=16, G2=2, G3=4, G4=16, GF=8):
    x = np.asarray(inputs["x"], np.float32)
    W = [np.asarray(inputs[f"W{i}"], np.float32) for i in (1, 2, 3, 4)]
    ei = [np.asarray(inputs[f"edge_index{i}"]).astype(np.int64) for i in range(4)]
    u = [np.asarray(inputs[f"unpool{i}"]).astype(np.int64) for i in (1, 2, 3, 4)]
    n = [x.shape[0], len(u[0]), len(u[1]), len(u[2])]
    nout = len(u[3])
    dis = [make_dis(ei[l], n[l]) for l in range(4)]

    # L1: table rows = original x ids (dis1 folded into table)
    L1 = plan_agg(ei[0][1], ei[0][0], n[0], self_row=np.arange(n[0]), G=G1,
                  slot_weight=np.ones(ei[0].shape[1], np.float32),
                  self_weight=np.ones(n[0], np.float32), col_budget=128)
    s1 = L1["sigma"]
    # L2: rows in T2 (=A1, global new order), composed via u1
    L2 = plan_agg(ei[1][1], s1[u[0][ei[1][0]]], n[1], self_row=s1[u[0]], G=G2,
                  slot_weight=dis[1][ei[1][0]], self_weight=dis[1])
    s2 = L2["sigma"]
    # L3: rows in T3 (=B2 = A2@W3), composed via u2
    L3 = plan_agg(ei[2][1], s2[u[1][ei[2][0]]], n[2], self_row=s2[u[1]], G=G3,
                  slot_weight=dis[2][ei[2][0]], self_weight=dis[2])
    s3 = L3["sigma"]
    # L4: rows in T4 (=B3 = A3@W4), composed via u3
    L4 = plan_agg(ei[3][1], s3[u[2][ei[3][0]]], n[3], self_row=s3[u[2]], G=G4,
                  slot_weight=dis[3][ei[3][0]], self_weight=dis[3],
                  col_budget=128)
    s4 = L4["sigma"]

    # dis-dst columns (new-local order per core)
    dis_dst = []
    for l, L in enumerate((L1, L2, L3, L4)):
        sh = n[l] // NC
        dis_dst.append(tile_cols(
            [dis[l][c * sh + L["perm"][c]] for c in range(NC)], L["shardP"]))

    # per-layer packing K (nodes per 256B+ table row, bf16)
    Ks = {"1": 16, "2": 4, "3": 8, "4": 16, "F": 32}
    Fs = {"1": 4, "2": 32, "3": 32, "4": 4, "F": 4}

    # final unpool is applied on the host during unshard/reassembly
    shf = nout // NC
    shfP = pad_to(shf, P)
    CF = shfP // P
    frows = s4[u[3]]

    meta = dict(
        n=n, nout=nout, shf=shf, shfP=shfP, CF=CF, Ks=Ks, Fs=Fs,
        L1=dict(groups=L1["groups"], shardP=L1["shardP"], C=L1["Ctot"],
                fout=32, wmat="w1", relu=True),
        L2=dict(groups=L2["groups"], shardP=L2["shardP"], C=L2["Ctot"],
                fout=64, wmat="w2", relu=True),
        L3=dict(groups=L3["groups"], shardP=L3["shardP"], C=L3["Ctot"],
                fout=32, wmat=None, relu=True),
        L4=dict(groups=L4["groups"], shardP=L4["shardP"], C=L4["Ctot"],
                fout=4, wmat=None, relu=True),
        T1_rows=pad_to(n[0], Ks["1"]) // Ks["1"],
        T2_rows=NC * L1["shardP"] // Ks["2"],
        T3_rows=NC * L2["shardP"] // Ks["3"],
        T4_rows=NC * L3["shardP"] // Ks["4"],
        T5_rows=NC * L4["shardP"] // Ks["F"],
    )

    # ---- per-core inputs ----
    # T1: x * dis1, padded to 4 feats, bf16, row-major [V,4] (viewed packed)
    V1 = pad_to(n[0], Ks["1"])
    T1 = np.zeros((V1, 4), np.float32)
    T1[:n[0], :3] = x * dis[0][:, None]

    W1p = np.zeros((4, 32), np.float32); W1p[:3] = W[0]
    W4p = np.zeros((32, 4), np.float32); W4p[:, :3] = W[3]

    def blkdiag(Wm, G):
        fi, fo = Wm.shape
        B = np.zeros((G * fi, G * fo), np.float32)
        for g in range(G):
            B[g * fi:(g + 1) * fi, g * fo:(g + 1) * fo] = Wm
        return B

    ident = np.eye(P, dtype=np.float32)
    qio = np.tile(np.arange(32, dtype=np.float32)[None, :], (128, 1)).astype(ml_dtypes.bfloat16)

    in_maps = []
    for c in range(NC):
        i1, q1 = split_kq(L1["idx"][c], Ks["1"])
        i2, q2 = split_kq(L2["idx"][c], Ks["2"])
        i3, q3 = split_kq(L3["idx"][c], Ks["3"])
        i4, q4 = split_kq(L4["idx"][c], Ks["4"])
        m = {
            "T1": T1, "ident": ident, "qio": qio,
            "w1blk": blkdiag(W1p, G1), "w2blk": blkdiag(W[1], G2),
            "w3blk": blkdiag(W[2], G2),
            "idx1": wrap16(i1), "q1": q1.astype(ml_dtypes.bfloat16),
            "wg1": L1["w"][c].astype(np.float32), "dis1d": dis_dst[0][c],
            "idx2": wrap16(i2), "q2": q2.astype(ml_dtypes.bfloat16),
            "wg2": L2["w"][c].astype(ml_dtypes.bfloat16), "dis2d": dis_dst[1][c],
            "idx3": wrap16(i3), "q3": q3.astype(ml_dtypes.bfloat16),
            "wg3": L3["w"][c].astype(ml_dtypes.bfloat16), "dis3d": dis_dst[2][c],
            "idx4": wrap16(i4), "q4": q4.astype(ml_dtypes.bfloat16),
            "wg4": L4["w"][c].astype(np.float32), "dis4d": dis_dst[3][c],
            "w4blk": blkdiag(W4p, G3),
        }
        in_maps.append(m)

    def reassemble(outs):
        A4f = np.concatenate([np.asarray(outs[c]["outbuf"], np.float32)
                              for c in range(NC)], axis=0)
        return np.ascontiguousarray(A4f[frows][:, :3].astype(np.float32))

    return meta, in_maps, reassemble


# ----------------------------------------------------------------------------
# device kernel
# ----------------------------------------------------------------------------

def build_kernel(meta):
    import concourse.bass as bass
    import concourse.mybir as mybir
    from concourse.bacc import Bacc
    from concourse.tile import TileContext
    from concourse import library_config

    f32 = mybir.dt.float32
    bf16 = mybir.dt.bfloat16
    i16 = mybir.dt.int16
    n = meta["n"]
    Ks, Fs = meta["Ks"], meta["Fs"]

    nc = Bacc("TRN2", target_bir_lowering=False, debug=False, num_devices=NC,
              num_swdge_queues=NQ)

    T1_d = nc.dram_tensor("T1", [meta["T1_rows"] * Ks["1"], 4], f32,
                          kind="ExternalInput")
    ident_d = nc.dram_tensor("ident", [P, P], f32, kind="ExternalInput")
    qio_d = nc.dram_tensor("qio", [P, 32], bf16, kind="ExternalInput")
    w1blk_d = nc.dram_tensor("w1blk", [16 * 4, 16 * 32], f32, kind="ExternalInput")
    w2blk_d = nc.dram_tensor("w2blk", [2 * 32, 2 * 64], f32, kind="ExternalInput")
    w3blk_d = nc.dram_tensor("w3blk", [2 * 64, 2 * 32], f32, kind="ExternalInput")
    w4blk_d = nc.dram_tensor("w4blk", [4 * 32, 4 * 4], f32, kind="ExternalInput")

    L1m, L2m, L3m, L4m = meta["L1"], meta["L2"], meta["L3"], meta["L4"]

    def grid_in(name, C, dt, scale=8):
        # idx grids are [P, C*128/16] int16; q/w grids [P, C]
        return nc.dram_tensor(name, [P, C * scale], dt, kind="ExternalInput")

    idx1 = grid_in("idx1", L1m["C"], i16); q1 = grid_in("q1", L1m["C"], bf16, 1)
    wg1 = grid_in("wg1", L1m["C"], f32, 1)
    dis1d = grid_in("dis1d", L1m["shardP"] // P, f32, 1)
    idx2 = grid_in("idx2", L2m["C"], i16); q2 = grid_in("q2", L2m["C"], bf16, 1)
    wg2 = grid_in("wg2", L2m["C"], bf16, 1)
    dis2d = grid_in("dis2d", L2m["shardP"] // P, f32, 1)
    idx3 = grid_in("idx3", L3m["C"], i16); q3 = grid_in("q3", L3m["C"], bf16, 1)
    wg3 = grid_in("wg3", L3m["C"], bf16, 1)
    dis3d = grid_in("dis3d", L3m["shardP"] // P, f32, 1)
    idx4 = grid_in("idx4", L4m["C"], i16); q4 = grid_in("q4", L4m["C"], bf16, 1)
    wg4 = grid_in("wg4", L4m["C"], f32, 1)
    dis4d = grid_in("dis4d", L4m["shardP"] // P, f32, 1)

    outbuf = nc.dram_tensor("outbuf", [L4m["shardP"], 4], bf16,
                            kind="ExternalOutput")
    rg = [list(range(NC))]

    qctr = [0]

    def next_q():
        q = qctr[0] % NQ
        qctr[0] += 1
        return q

    with TileContext(nc) as tc:
        nc.gpsimd.load_library(library_config.mlp)
        with (
            tc.tile_pool(name="dramp", bufs=1, space="DRAM") as dramp,
            tc.tile_pool(name="consts", bufs=1) as constp,
            tc.tile_pool(name="idxp", bufs=5) as idxp,
            tc.tile_pool(name="gath", bufs=5) as gathp,
            tc.tile_pool(name="work", bufs=3) as workp,
            tc.tile_pool(name="outp", bufs=2) as outp,
            tc.tile_pool(name="psumT", bufs=2, space="PSUM") as psumTp,
            tc.tile_pool(name="psumM", bufs=2, space="PSUM") as psumMp,
        ):
            # persistent DRAM intermediates (bf16 tables)
            A1sh = dramp.tile([L1m["shardP"], 32], bf16)
            T2f = dramp.tile([NC * L1m["shardP"], 32], bf16, addr_space="Shared")
            B2sh = dramp.tile([L2m["shardP"], 32], bf16)
            T3f = dramp.tile([NC * L2m["shardP"], 32], bf16, addr_space="Shared")
            B3sh = dramp.tile([L3m["shardP"], 4], f32)
            T4f = dramp.tile([NC * L3m["shardP"], 4], f32, addr_space="Shared")

            # constants
            ident = constp.tile([P, P], f32)
            nc.sync.dma_start(out=ident[:], in_=ident_d[:, :])
            qio16 = constp.tile([P, 32], bf16)
            nc.sync.dma_start(out=qio16[:], in_=qio_d[:, :])
            w1b = constp.tile([64, 16 * 32], f32)
            nc.sync.dma_start(out=w1b[:], in_=w1blk_d[:, :])
            w2b = constp.tile([64, 2 * 64], f32)
            nc.sync.dma_start(out=w2b[:], in_=w2blk_d[:, :])
            w3b = constp.tile([2 * 64, 2 * 32], f32)
            nc.sync.dma_start(out=w3b[:], in_=w3blk_d[:, :])
            w4b = constp.tile([4 * 32, 4 * 4], f32)
            nc.sync.dma_start(out=w4b[:], in_=w4blk_d[:, :])

            def agg_phase(lm, K, F, idx_d, q_d, w_d, disd_d, table_view,
                          wblk, out_dram, out_f, second=None, tag="",
                          tdt=bf16):
                """One GCN layer aggregation over the padded-CSR grid.

                table_view: DRAM AP [rows, K*F] bf16.
                wblk: None -> identity collapse (gathered feats are final);
                      else (tile, Gfi, Gfo) block-diag matmul after reduce.
                second: optional (w3b-style tile, fi, fo) fused second
                      matmul producing out rows (for B2).
                """
                t0 = 0
                col = 0
                for gi, (Gg, kt) in enumerate(lm["groups"]):
                    ncols = Gg * kt
                    nslots = ncols * P
                    # gathered tile [P, ncols, K*F] bf16 (slot i -> i%128,i//128)
                    gt = gathp.tile([P, ncols * K * F], tdt, tag="g",
                                    name=f"g{tag}_{gi}")
                    # gather in chunks of MAXIDX slots (=MAXIDX/128 cols)
                    ccols = MAXIDX // P
                    for c0 in range(0, ncols, ccols):
                        cw = min(ccols, ncols - c0)
                        idxt = idxp.tile([P, cw * P // 16], i16, tag="i",
                                         name=f"i{tag}_{gi}_{c0}")
                        nc.sync.dma_start(
                            out=idxt[:],
                            in_=idx_d[:, (col + c0) * 8:(col + c0 + cw) * 8])
                        nc.gpsimd.dma_gather(
                            out_ap=gt[:, c0 * K * F:(c0 + cw) * K * F]
                                .rearrange("p (m e) -> p m e", e=K * F),
                            in_ap=table_view,
                            idxs_ap=idxt[:],
                            num_idxs=cw * P, num_idxs_reg=cw * P,
                            elem_size=K * F,
                            single_packet=False, queue_num=next_q())
                    # subpos mask: ind[p, c, K] = (q[p,c] == qio[K])
                    qt = idxp.tile([P, ncols], bf16, tag="q",
                                   name=f"q{tag}_{gi}")
                    nc.sync.dma_start(out=qt[:], in_=q_d[:, col:col + ncols])
                    wt = idxp.tile([P, ncols], tdt, tag="w",
                                   name=f"w{tag}_{gi}")
                    nc.sync.dma_start(out=wt[:], in_=w_d[:, col:col + ncols])
                    ind = workp.tile([P, ncols * K], tdt, tag="n",
                                     name=f"n{tag}_{gi}")
                    nc.vector.tensor_tensor(
                        out=ind[:].rearrange("p (c k) -> p c k", k=K),
                        in0=qt[:].to_broadcast([P, ncols, K]),
                        in1=qio16[:, :K].rearrange("p (o k) -> p o k", o=1)
                            .to_broadcast([P, ncols, K]),
                        op=mybir.AluOpType.is_equal)
                    # fold w: mw = ind * w
                    nc.vector.tensor_tensor(
                        out=ind[:].rearrange("p (c k) -> p c k", k=K),
                        in0=ind[:].rearrange("p (c k) -> p c k", k=K),
                        in1=wt[:].to_broadcast([P, ncols, K]),
                        op=mybir.AluOpType.mult)
                    # apply to gathered rows
                    nc.vector.tensor_tensor(
                        out=gt[:].rearrange("p (c k f) -> p c k f", k=K, f=F),
                        in0=gt[:].rearrange("p (c k f) -> p c k f", k=K, f=F),
                        in1=ind[:].rearrange("p (c k) -> p c k", k=K)
                            .to_broadcast([P, ncols, K, F]),
                        op=mybir.AluOpType.mult)
                    # segment reduce over (kt*K) keeping F
                    S = workp.tile([P, Gg * F], f32, tag="S",
                                   name=f"S{tag}_{gi}")
                    nc.vector.tensor_reduce(
                        out=S[:].rearrange("p (g f) -> p g f", f=F),
                        in_=gt[:].rearrange("p (g x f) -> p g f x",
                                            g=Gg, x=kt * K),
                        axis=mybir.AxisListType.X, op=mybir.AluOpType.add)
                    # dis_dst scale
                    dcol = idxp.tile([P, Gg], f32, tag="d",
                                     name=f"d{tag}_{gi}")
                    nc.sync.dma_start(out=dcol[:], in_=disd_d[:, t0:t0 + Gg])
                    nc.vector.tensor_tensor(
                        out=S[:].rearrange("p (g f) -> p g f", f=F),
                        in0=S[:].rearrange("p (g f) -> p g f", f=F),
                        in1=dcol[:].to_broadcast([P, Gg, F]),
                        op=mybir.AluOpType.mult)
                    if wblk is not None:
                        wtile, gfi, gfo = wblk
                        pT = psumTp.tile([Gg * gfi, P], f32, tag="pT",
                                         name=f"pT{tag}_{gi}")
                        nc.tensor.transpose(out=pT[:], in_=S[:],
                                            identity=ident[:])
                        ST = workp.tile([Gg * gfi, P], f32, tag="ST",
                                        name=f"ST{tag}_{gi}")
                        nc.scalar.copy(out=ST[:], in_=pT[:])
                        pM = psumMp.tile([P, Gg * gfo], f32, tag="pM",
                                         name=f"pM{tag}_{gi}")
                        nc.tensor.matmul(out=pM[:], lhsT=ST[:],
                                         rhs=wtile[:Gg * gfi, :Gg * gfo],
                                         start=True, stop=True)
                        src_ap = pM
                        fo = gfo
                    else:
                        src_ap = S
                        fo = F
                    at = outp.tile([P, Gg * fo], f32, tag="A",
                                   name=f"A{tag}_{gi}")
                    nc.scalar.activation(
                        out=at[:], in_=src_ap[:],
                        func=mybir.ActivationFunctionType.Relu)
                    if second is None:
                        ab = outp.tile([P, Gg * fo], bf16, tag="Ab",
                                       name=f"Ab{tag}_{gi}")
                        nc.vector.tensor_copy(out=ab[:], in_=at[:])
                        nc.sync.dma_start(
                            out=out_dram[t0 * P:(t0 + Gg) * P, :].rearrange(
                                "(g p) f -> p g f", p=P),
                            in_=ab[:])
                    else:
                        stile, sfi, sfo = second
                        pT2 = psumTp.tile([Gg * sfi, P], f32, tag="pT",
                                          name=f"pT2{tag}_{gi}")
                        nc.tensor.transpose(out=pT2[:], in_=at[:],
                                            identity=ident[:])
                        ST2 = workp.tile([Gg * sfi, P], f32, tag="ST2",
                                         name=f"ST2{tag}_{gi}")
                        nc.scalar.copy(out=ST2[:], in_=pT2[:])
                        pM2 = psumMp.tile([P, Gg * sfo], f32, tag="pM",
                                          name=f"pM2{tag}_{gi}")
                        nc.tensor.matmul(out=pM2[:], lhsT=ST2[:],
                                         rhs=stile[:Gg * sfi, :Gg * sfo],
                                         start=True, stop=True)
                        ab = outp.tile([P, Gg * sfo],
                                       f32 if tag == "3" else bf16, tag="Ab2",
                                       name=f"Ab2{tag}_{gi}")
                        nc.vector.tensor_copy(out=ab[:], in_=pM2[:])
                        nc.sync.dma_start(
                            out=out_dram[t0 * P:(t0 + Gg) * P, :].rearrange(
                                "(g p) f -> p g f", p=P),
                            in_=ab[:])
                    t0 += Gg
                    col += ncols

            # ---------------- L1 ----------------
            agg_phase(L1m, Ks["1"], Fs["1"], idx1, q1, wg1, dis1d,
                      T1_d[:, :].rearrange("(r k) f -> r (k f)", k=Ks["1"]),
                      (w1b, 4, 32), A1sh[:], 32, tag="1", tdt=f32)
            nc.gpsimd.collective_compute(
                "AllGather", mybir.AluOpType.bypass, replica_groups=rg,
                ins=[A1sh[:]], outs=[T2f[:]])
            # ---------------- L2 (fused B2 = relu(...)@W3) ----------------
            agg_phase(L2m, Ks["2"], Fs["2"], idx2, q2, wg2, dis2d,
                      T2f[:].rearrange("(r k) f -> r (k f)", k=Ks["2"]),
                      (w2b, 32, 64), B2sh[:], 32, second=(w3b, 64, 32),
                      tag="2")
            nc.gpsimd.collective_compute(
                "AllGather", mybir.AluOpType.bypass, replica_groups=rg,
                ins=[B2sh[:]], outs=[T3f[:]])
            # ---------------- L3 (identity collapse; B3 = relu@W4) --------
            agg_phase(L3m, Ks["3"], Fs["3"], idx3, q3, wg3, dis3d,
                      T3f[:].rearrange("(r k) f -> r (k f)", k=Ks["3"]),
                      None, B3sh[:], 4, second=(w4b, 32, 4), tag="3")
            nc.gpsimd.collective_compute(
                "AllGather", mybir.AluOpType.bypass, replica_groups=rg,
                ins=[B3sh[:]], outs=[T4f[:]])
            # ---------------- L4 ----------------
            agg_phase(L4m, Ks["4"], Fs["4"], idx4, q4, wg4, dis4d,
                      T4f[:].rearrange("(r k) f -> r (k f)", k=Ks["4"]),
                      None, outbuf[:, :], 4, tag="4", tdt=f32)

    nc.finalize()
    return nc


# ----------------------------------------------------------------------------
# PJRT runner (persistent compiled callable, device-resident inputs)
# ----------------------------------------------------------------------------
import numpy as np, time
import jax
import jax.numpy as jnp
from jax.sharding import Mesh, PartitionSpec, NamedSharding
from jax.experimental.shard_map import shard_map
from concourse import mybir
from concourse.bass2jax import _bass_exec_p, partition_id_tensor, install_neuronx_cc_hook


def make_runner(nc, n_cores=8):
    install_neuronx_cc_hook()
    partition_name = nc.partition_id_tensor.name if nc.partition_id_tensor else None
    in_names, out_names, out_avals = [], [], []
    for alloc in nc.m.functions[0].allocations:
        if not isinstance(alloc, mybir.MemoryLocationSet):
            continue
        name = alloc.memorylocations[0].name
        if alloc.kind == "ExternalInput":
            if name != partition_name:
                in_names.append(name)
        elif alloc.kind == "ExternalOutput":
            out_names.append(name)
            out_avals.append(jax.core.ShapedArray(
                tuple(alloc.tensor_shape), mybir.dt.np(alloc.dtype)))
    n_params = len(in_names)
    all_in_names = list(in_names) + list(out_names)
    if partition_name is not None:
        all_in_names.append(partition_name)

    def _body(*args):
        operands = list(args)
        if partition_name is not None:
            operands.append(partition_id_tensor())
        outs = _bass_exec_p.bind(
            *operands,
            out_avals=tuple(out_avals), in_names=tuple(all_in_names),
            out_names=tuple(out_names), lowering_input_output_aliases=(),
            sim_require_finite=False, sim_require_nnan=False, nc=nc)
        return tuple(outs)

    devices = jax.devices()[:n_cores]
    mesh = Mesh(np.asarray(devices), ("core",))
    n_outs = len(out_avals)
    in_specs = (PartitionSpec("core"),) * (n_params + n_outs)
    out_specs = (PartitionSpec("core"),) * len(out_names)
    sharded = jax.jit(shard_map(_body, mesh=mesh, in_specs=in_specs,
                                out_specs=out_specs, check_rep=False),
                      keep_unused=True)
    sharding = NamedSharding(mesh, PartitionSpec("core"))

    state = {}

    def prepare(in_maps):
        per_core = [[np.asarray(m[name]) for name in in_names] for m in in_maps]
        concat_in = [np.concatenate([per_core[c][i] for c in range(n_cores)], axis=0)
                     for i in range(n_params)]
        zeros = [np.zeros((n_cores * av.shape[0], *av.shape[1:]), av.dtype)
                 for av in out_avals]
        state["dev_in"] = [jax.device_put(a, sharding) for a in concat_in + zeros]
        jax.block_until_ready(state["dev_in"])

    def run():
        out = jax.block_until_ready(sharded(*state["dev_in"]))
        return out

    def fetch(out_arrs):
        return [
            {name: np.asarray(out_arrs[i]).reshape(n_cores, *out_avals[i].shape)[c]
             for i, name in enumerate(out_names)}
            for c in range(n_cores)
        ]

    return prepare, run, fetch



_CACHE = {}


def kernel(**inputs):
    for b in ("b1", "b2", "b3", "b4"):
        if b in inputs:
            assert not np.asarray(inputs[b]).any()
    meta, in_maps, reassemble = prep(inputs)
    if "k" not in _CACHE:
        nc = build_kernel(meta)
        _CACHE["k"] = make_runner(nc)
    prepare, run, fetch = _CACHE["k"]
    prepare(in_maps)
    outs = fetch(run())
    return reassemble(outs).astype(np.float32)
